# revision 24
# baseline (speedup 1.0000x reference)
"""Trainium2 Bass kernel for nn_Attention_54065048322573.

XCA/Restormer-style channel attention:
  q = dwconv3x3(conv1x1(high)); k,v = split(dwconv3x3(conv1x1(low)))
  q,k L2-normalized over space; attn = softmax((q@k^T)*temp); out = conv1x1(attn@v)

Strategy: spatial sharding over H (20 rows/core + 1-row halo); all compute
local except a 139KB AllReduce of per-head gram diagonals + norms.

v2 design (engine-balanced):
  - all-bf16 matmuls; bf16 input slabs (half the DMA of f32)
  - per (ptile, batch) processing of the whole 22-row slab
  - depthwise 3x3: 9 taps split across engines:
      5 taps as PE diagonal matmuls (psum; Act/DVE copies seed the acc)
      2 taps as DVE tensor_scalar_mul (4x) + tensor_tensor add (2x)
      2 taps as Pool tensor_scalar_mul + tensor_tensor add on DVE
  - proj fused into attention: M_g = A_g^T @ P_g per (batch, group);
    output is one matmul stage out = sum_g M_g^T @ v_g (removes the
    attn@v stage and its psum->sbuf copies)
  - gram transposes emitted before the v-taps so the AllReduce+softmax
    path starts early; softmax skips max-subtract (logits bounded by
    |temperature| since q,k are L2-normalized) and fuses exp+rowsum
  - transposed-chunk copies on DVE; conv psum->sbuf copies on Act
"""

import contextlib
import functools
import os
import sys

import numpy as np

for _p in ("/opt/trn_rl_repo", os.path.expanduser("~/.axon_site/_ro/trn_rl_repo")):
    if os.path.isdir(_p) and _p not in sys.path:
        sys.path.insert(0, _p)

import ml_dtypes  # noqa: E402

B, C, H, W = 4, 256, 160, 160
HEADS, CH = 8, 32
NCORES = 8
RPC = H // NCORES          # rows per core = 20
WE = W + 2                 # 162 padded width
SLABR = RPC + 2            # 22 slab rows (with halo)
INT = RPC * W              # 3200 interior cols per (ptile, b)

TAPS = [(dy, dx) for dy in (-1, 0, 1) for dx in (-1, 0, 1)]
PE_TAPS = (0, 2, 4, 6, 8)  # PE diagonal matmuls (psum accumulation)
DVE_TAPS = (1, 7)          # tsm on DVE + tt-add on DVE
POOL_TAPS = (3, 5)         # tsm on Pool + tt-add on DVE
ACT_TAPS = ()              # (Act keeps copies/norms instead)


# ---------------------------------------------------------------- host prep

def _prep_weights(q_w, q_dw_w, kv_w, kv_dw_w, proj_w, temperature):
    f32, bf16 = np.float32, ml_dtypes.bfloat16
    wq = np.ascontiguousarray(
        q_w[:, :, 0, 0].T.reshape(2, 128, 256), dtype=bf16)
    wkv = np.ascontiguousarray(
        kv_w[:, :, 0, 0].T.reshape(2, 128, 512), dtype=bf16)
    wproj = np.ascontiguousarray(
        proj_w[:, :, 0, 0].T.reshape(2, 128, 256), dtype=bf16)

    # depthwise taps; 6 channel-ptiles = [q0 q1 k0 k1 v0 v1]
    dwq = np.asarray(q_dw_w, f32)[:, 0].reshape(C, 9)
    dwkv = np.asarray(kv_dw_w, f32)[:, 0].reshape(2 * C, 9)
    dwall = np.concatenate([dwq, dwkv], axis=0)              # [768, 9]
    dwvec = np.ascontiguousarray(
        dwall.reshape(6, 128, 9).transpose(1, 0, 2))         # [128, 6, 9]
    npe = len(PE_TAPS)
    dwdiag = np.zeros((128, npe, 6, 128), dtype=bf16)
    for ti, t in enumerate(PE_TAPS):
        for pt in range(6):
            np.fill_diagonal(dwdiag[:, ti, pt, :],
                             dwall[pt * 128:(pt + 1) * 128, t])
    dwdiag = np.ascontiguousarray(dwdiag.reshape(128, npe * 6 * 128))

    tmpr = np.repeat(np.asarray(temperature, f32).reshape(HEADS), CH)
    tmpr = np.ascontiguousarray(tmpr.reshape(2, 128).T)      # [128, 2]

    ident = np.ascontiguousarray(np.eye(128, dtype=bf16))
    ones = np.ones((1, 128), dtype=f32)
    return dict(wq=wq, wkv=wkv, wproj=wproj, dwvec=dwvec, dwdiag=dwdiag,
                tmpr=tmpr, ident=ident, ones=ones)


def _prep_slabs(low, high):
    """Per-core input slabs [B, 2, 128, SLABR, WE] bf16 with zero halo/pad."""
    out = {}
    for name, x in (("high_s", high), ("low_s", low)):
        xp = np.zeros((B, C, H + 2, WE), dtype=ml_dtypes.bfloat16)
        xp[:, :, 1:-1, 1:W + 1] = np.asarray(x, dtype=ml_dtypes.bfloat16)
        out[name] = [
            np.ascontiguousarray(
                xp[:, :, RPC * i: RPC * i + SLABR, :].reshape(
                    B, 2, 128, SLABR, WE))
            for i in range(NCORES)
        ]
    return out


# ---------------------------------------------------------------- device build

@functools.lru_cache(maxsize=6)
def _build_nc(pe_taps=PE_TAPS, use_collective=True, loop_n=0):
    import concourse.bass as bass
    import concourse.mybir as mybir
    import concourse.tile as tile
    from concourse import bacc

    f32, bf16 = mybir.dt.float32, mybir.dt.bfloat16
    AOP = mybir.AluOpType
    AF = mybir.ActivationFunctionType

    npe = len(pe_taps)

    nc = bacc.Bacc("TRN2", target_bir_lowering=False, debug=False,
                   num_devices=NCORES)

    hi_d = nc.dram_tensor("high_s", [B, 2, 128, SLABR, WE], bf16,
                          kind="ExternalInput").ap()
    lo_d = nc.dram_tensor("low_s", [B, 2, 128, SLABR, WE], bf16,
                          kind="ExternalInput").ap()
    wq_d = nc.dram_tensor("wq", [2, 128, 256], bf16, kind="ExternalInput").ap()
    wkv_d = nc.dram_tensor("wkv", [2, 128, 512], bf16,
                           kind="ExternalInput").ap()
    wproj_d = nc.dram_tensor("wproj", [2, 128, 256], bf16,
                             kind="ExternalInput").ap()
    dwvec_d = nc.dram_tensor("dwvec", [128, 6, 9], f32,
                             kind="ExternalInput").ap()
    dwdiag_d = nc.dram_tensor("dwdiag", [128, npe * 6 * 128], bf16,
                              kind="ExternalInput").ap()
    tmpr_d = nc.dram_tensor("tmpr", [128, 2], f32, kind="ExternalInput").ap()
    ident_d = nc.dram_tensor("ident", [128, 128], bf16,
                             kind="ExternalInput").ap()
    ones_d = nc.dram_tensor("ones", [1, 128], f32,
                            kind="ExternalInput").ap()
    out_d = nc.dram_tensor("out", [B, 2, 128, INT], f32,
                           kind="ExternalOutput").ap()
    cc_in = nc.dram_tensor("cc_in", [B, 2, 128, 34], f32).ap()
    cc_out = nc.dram_tensor("cc_out", [B, 2, 128, 34], f32,
                            addr_space="Shared").ap()
    rk_dram = nc.dram_tensor("rk_tmp", [B, 256], f32).ap()

    with tile.TileContext(nc) as tc, contextlib.ExitStack() as ctx:
        ec = ctx.enter_context
        if loop_n:
            ec(tc.For_i(0, loop_n, 1))
        consts = ec(tc.tile_pool(name="consts", bufs=1))
        inp_p = ec(tc.tile_pool(name="inp", bufs=1))
        qkv_p = ec(tc.tile_pool(name="qkv", bufs=1))
        acc_p = ec(tc.tile_pool(name="acc", bufs=2))
        vt_p = ec(tc.tile_pool(name="vt", bufs=2))
        tmp_p = ec(tc.tile_pool(name="tmp", bufs=2))
        nb_p = ec(tc.tile_pool(name="nb", bufs=2))
        tsb_p = ec(tc.tile_pool(name="tsb", bufs=4))
        small_p = ec(tc.tile_pool(name="small", bufs=2))
        att_p = ec(tc.tile_pool(name="att", bufs=2))
        m_p = ec(tc.tile_pool(name="mw", bufs=2))
        osb_p = ec(tc.tile_pool(name="osb", bufs=2))

        cv_ps = ec(tc.tile_pool(name="cvps", bufs=2, space="PSUM"))
        dw_ps = ec(tc.tile_pool(name="dwps", bufs=2, space="PSUM"))
        tr_ps = ec(tc.tile_pool(name="trps", bufs=1, space="PSUM"))
        gm_ps = ec(tc.tile_pool(name="gmps", bufs=1, space="PSUM"))
        mm_ps = ec(tc.tile_pool(name="mmps", bufs=2, space="PSUM"))

        # ---- constants
        wq_sb = [consts.tile([128, 256], bf16, tag=f"wq{k}", name=f"wq{k}")
                 for k in range(2)]
        wkv_sb = [consts.tile([128, 512], bf16, tag=f"wkv{k}", name=f"wkv{k}")
                  for k in range(2)]
        wproj_sb = [consts.tile([128, 256], bf16, tag=f"wp{k}", name=f"wp{k}")
                    for k in range(2)]
        for k in range(2):
            nc.sync.dma_start(out=wq_sb[k][:], in_=wq_d[k])
            nc.sync.dma_start(out=wkv_sb[k][:], in_=wkv_d[k])
            nc.sync.dma_start(out=wproj_sb[k][:], in_=wproj_d[k])
        dwvec_sb = consts.tile([128, 6, 9], f32, tag="dwvec", name="dwvec")
        nc.sync.dma_start(out=dwvec_sb[:], in_=dwvec_d)
        dwdiag_sb = consts.tile([128, npe * 6 * 128], bf16, tag="dwdiag",
                                name="dwdiag")
        nc.sync.dma_start(out=dwdiag_sb[:], in_=dwdiag_d)
        tmpr_sb = consts.tile([128, 2], f32, tag="tmpr", name="tmpr")
        nc.sync.dma_start(out=tmpr_sb[:], in_=tmpr_d)
        ident_sb = consts.tile([128, 128], bf16, tag="ident", name="ident")
        nc.sync.dma_start(out=ident_sb[:], in_=ident_d)
        ones_sb = consts.tile([1, 128], f32, tag="ones", name="ones")
        nc.sync.dma_start(out=ones_sb[:], in_=ones_d)

        def dwd(ti, pt):
            i = ti * 6 + pt
            return dwdiag_sb[:, i * 128:(i + 1) * 128]

        vt_tiles = {}

        def emit_phase3(b):
            red2 = att_p.tile([128, 2, 34], f32, tag="red2", name="red2")
            nc.sync.dma_start(out=red2[:],
                              in_=cc_out[b].rearrange("g p e -> p g e"))
            red = {g: red2[:, g, :] for g in range(2)}

            # k norms as [1,256] row; sqrt+recip; partition-broadcast AP
            knrow = att_p.tile([1, 256], f32, tag="knrow", name="knrow")
            base = cc_out[b]
            kn_flat = bass.AP(tensor=base.tensor, offset=base.offset + 33,
                              ap=[[0, 1], [34, 256]])
            nc.sync.dma_start(out=knrow[:], in_=kn_flat)
            krt = att_p.tile([1, 256], f32, tag="krt", name="krt")
            nc.scalar.activation(out=krt[:], in_=knrow[:], func=AF.Sqrt)
            rkrow = att_p.tile([1, 256], f32, tag="rkrow", name="rkrow")
            nc.vector.reciprocal(rkrow[:], krt[:])

            # broadcast rkrow across partitions via PE outer product
            rkb = mm_ps.tile([128, 256], f32, tag="mm", name="rkb")
            nc.tensor.matmul(rkb[:], lhsT=ones_sb[:], rhs=rkrow[:],
                             start=True, stop=True)

            def rk_bc(off):
                return rkb[:, off:off + 32]

            rqt = {}
            for g in range(2):
                qn = att_p.tile([128, 1], f32, tag=f"qn{g}", name=f"qn{g}")
                nc.scalar.activation(out=qn[:], in_=red[g][:, 32:33],
                                     func=AF.Sqrt)
                rq = att_p.tile([128, 1], f32, tag=f"rq{g}", name=f"rq{g}")
                nc.vector.reciprocal(rq[:], qn[:])
                rqt[g] = att_p.tile([128, 1], f32, tag=f"rqt{g}",
                                    name=f"rqt{g}")
                nc.vector.tensor_tensor(out=rqt[g][:], in0=rq[:],
                                        in1=tmpr_sb[:, g:g + 1], op=AOP.mult)

            # logits bounded by |temperature| (q,k L2-normalized), so exp is
            # overflow-safe without the max-subtract; exp+rowsum fused
            lg = att_p.tile([128, 2, 32], f32, tag="lg", name="lg")
            for g in range(2):
                for h in range(4):
                    sl = slice(h * 32, (h + 1) * 32)
                    nc.vector.scalar_tensor_tensor(
                        out=lg[sl, g, :], in0=red[g][sl, 0:32],
                        scalar=rqt[g][sl, :],
                        in1=rk_bc(g * 128 + h * 32)[sl, :],
                        op0=AOP.mult, op1=AOP.mult)
            ee = att_p.tile([128, 2, 32], f32, tag="ee", name="ee")
            ssum = att_p.tile([128, 2], f32, tag="ssum", name="ssum")
            for g in range(2):
                nc.scalar.activation(out=ee[:, g, :], in_=lg[:, g, :],
                                     func=AF.Exp,
                                     accum_out=ssum[:, g:g + 1])
            rs = att_p.tile([128, 2], f32, tag="rs", name="rs")
            nc.vector.reciprocal(rs[:], ssum[:])
            rsb = bass.AP(tensor=rs.tensor, offset=rs.offset,
                          ap=[list(rs.ap[0]), [1, 2], [0, 32]])
            nc.vector.tensor_tensor(out=ee[:], in0=ee[:], in1=rsb,
                                    op=AOP.mult)

            # A blocks (un-transposed) -> M_g = A_g @ P_g  [128 d, 256 o]
            mg = {}
            for g in range(2):
                abd = att_p.tile([128, 128], bf16, tag=f"abd{g}",
                                 name=f"abd{g}")
                nc.vector.memset(abd[:], 0.0)
                for h in range(4):
                    sl = slice(h * 32, (h + 1) * 32)
                    nc.scalar.copy(out=abd[sl, sl], in_=ee[sl, g, :])
                mps = mm_ps.tile([128, 256], f32, tag="mm", name="mb")
                nc.tensor.matmul(mps[:], lhsT=abd[:], rhs=wproj_sb[g][:],
                                 start=True, stop=True)
                mg[g] = m_p.tile([128, 256], bf16, tag=f"mg{g}",
                                 name=f"mg{g}")
                nc.scalar.copy(out=mg[g][:], in_=mps[:])

            # fused (attn+proj): out[oc] = sum_g M_g[:, oc]^T @ v_g
            for chk in range(8):
                c0 = chk * 400
                obs = osb_p.tile([128, 2, 400], f32, tag="obs", name="obs")
                for oc in range(2):
                    ps = mm_ps.tile([128, 400], f32, tag="mm", name="mm")
                    for g in range(2):
                        nc.tensor.matmul(
                            ps[:],
                            lhsT=mg[g][:, oc * 128:(oc + 1) * 128],
                            rhs=vt_tiles[(b, g)][:, c0:c0 + 400],
                            start=(g == 0), stop=(g == 1))
                    if oc == 0:
                        nc.scalar.copy(out=obs[:, oc, :], in_=ps[:])
                    else:
                        nc.vector.tensor_copy(obs[:, oc, :], ps[:])
                for oc in range(2):
                    nc.sync.dma_start(out=out_d[b, oc, :, c0:c0 + 400],
                                      in_=obs[:, oc, :])

        # =================== phase 1: conv + dwconv + gram ===================
        row_chunks_conv = [(r0, min(3, SLABR - r0)) for r0 in range(0, SLABR, 3)]
        row_chunks_dw = [(r0, min(3, RPC - r0)) for r0 in range(0, RPC, 3)]

        for b in range(B):
            # ---- conv1x1 into 6 slab tiles [128, SLABR, WE] bf16
            slabs = {}
            for src_d, names, wsb in (
                    (hi_d, ("q0", "q1"), wq_sb),
                    (lo_d, ("k0", "k1", "v0", "v1"), wkv_sb)):
                ins = []
                for kc in range(2):
                    ti = inp_p.tile([128, SLABR, WE], bf16,
                                    tag=f"in{names[0][0]}{kc}",
                                    name=f"in{names[0][0]}{kc}")
                    nc.sync.dma_start(out=ti[:], in_=src_d[b, kc])
                    ins.append(ti)
                for o, nm in enumerate(names):
                    slab = qkv_p.tile([128, SLABR, WE], bf16, tag=nm, name=nm)
                    slabs[nm] = slab
                    for ci, (r0, nr) in enumerate(row_chunks_conv):
                        ps = cv_ps.tile([128, 3, WE], f32, tag="cv", name="cv")
                        for kc in range(2):
                            nc.tensor.matmul(
                                ps[:, :nr, :],
                                lhsT=wsb[kc][:, o * 128:(o + 1) * 128],
                                rhs=ins[kc][:, r0:r0 + nr, :],
                                start=(kc == 0), stop=(kc == 1))
                        nc.scalar.copy(out=slab[:, r0:r0 + nr, :],
                                       in_=ps[:, :nr, :])

            # ---- depthwise 3x3 into acc tiles [128, RPC, W] bf16
            nb = {}
            accs = {}

            def emit_dw(ptg, nm):
                srcv = slabs[nm][:]

                def tap_ap(t, r0=0, nr=RPC, _s=srcv):
                    dy, dx = TAPS[t]
                    return _s[:, r0 + 1 + dy:r0 + 1 + dy + nr,
                              1 + dx:1 + dx + W]

                if ptg >= 4:
                    acc = vt_p.tile([128, INT], bf16, tag=f"vt{ptg - 4}",
                                    name=f"vt{b}_{ptg - 4}")
                    vt_tiles[(b, ptg - 4)] = acc
                else:
                    acc = acc_p.tile([128, INT], bf16, tag=f"a{ptg}",
                                     name=f"a{ptg}")
                accv = acc[:].rearrange("p (r w) -> p r w", r=RPC)

                # PE taps -> psum chunks; copies seed the accumulator
                for di, (r0, nr) in enumerate(row_chunks_dw):
                    ps = dw_ps.tile([128, 3, W], f32, tag="dw", name="dw")
                    for ti_, t in enumerate(pe_taps):
                        nc.tensor.matmul(
                            ps[:, :nr, :], lhsT=dwd(ti_, ptg),
                            rhs=tap_ap(t, r0=r0, nr=nr),
                            start=(ti_ == 0), stop=(ti_ == npe - 1))
                    if di % 3 == 2:
                        nc.vector.tensor_copy(accv[:, r0:r0 + nr, :],
                                              ps[:, :nr, :])
                    else:
                        nc.scalar.copy(out=accv[:, r0:r0 + nr, :],
                                       in_=ps[:, :nr, :])

                def wv(t):
                    return dwvec_sb[:, ptg, t:t + 1]

                # non-PE taps: scale on DVE/Pool, accumulate on DVE
                for t in DVE_TAPS + POOL_TAPS + ACT_TAPS:
                    tmp = tmp_p.tile([128, INT], bf16, tag="ttmp",
                                     name=f"tmp{t}", bufs=3)
                    tmpv = tmp[:].rearrange("p (r w) -> p r w", r=RPC)
                    if t in POOL_TAPS:
                        nc.gpsimd.tensor_scalar_mul(tmpv, tap_ap(t), wv(t))
                    elif t in ACT_TAPS:
                        nc.scalar.activation(out=tmpv, in_=tap_ap(t),
                                             func=AF.Identity, scale=wv(t))
                    else:
                        nc.vector.tensor_scalar_mul(tmpv, tap_ap(t), wv(t))
                    nc.vector.tensor_tensor(out=acc[:], in0=acc[:],
                                            in1=tmp[:], op=AOP.add)

                # ---- norms for q,k ptiles
                if ptg < 4:
                    sc = tmp_p.tile([128, INT], bf16, tag="ttmp", name="sq",
                                    bufs=3)
                    nbt = nb_p.tile([128, 1], f32, tag=f"nb{ptg}",
                                    name=f"nb{ptg}")
                    nb[ptg] = nbt
                    nc.scalar.activation(out=sc[:], in_=acc[:],
                                         func=AF.Square, accum_out=nbt[:])
                    accs[ptg] = acc

            for ptg, nm in enumerate(("q0", "q1", "k0", "k1")):
                emit_dw(ptg, nm)

            # ---- transposes + gram accumulation (25 chunks of 128)
            # (before v-taps so the gram path starts as early as possible)
            gram = gm_ps.tile([128, 2, 128], f32, tag="gram", name="gram")
            nchunks = INT // 128
            for chi in range(nchunks):
                c0 = chi * 128
                tt = tsb_p.tile([128, 4, 128], bf16, tag="tqk", name="tqk")
                tp = tr_ps.tile([128, 4, 128], bf16, tag="tp", name="tp")
                for pi in range(4):
                    nc.tensor.transpose(
                        tp[:, pi, :], in_=accs[pi][:, c0:c0 + 128],
                        identity=ident_sb[:])
                nc.vector.tensor_copy(tt[:], tp[:])
                for g in range(2):
                    nc.tensor.matmul(
                        gram[:, g, :], lhsT=tt[:, g, :], rhs=tt[:, 2 + g, :],
                        start=(chi == 0), stop=(chi == nchunks - 1))

            for ptg, nm in enumerate(("v0", "v1")):
                emit_dw(4 + ptg, nm)

            # ---- stage gram diagonals + norms
            for g in range(2):
                sg = small_p.tile([128, 34], f32, tag="stage", name="stage")
                for h in range(4):
                    sl = slice(h * 32, (h + 1) * 32)
                    nc.scalar.copy(out=sg[sl, 0:32], in_=gram[sl, g, sl])
                nc.vector.tensor_copy(sg[:, 32:33], nb[g][:])      # q norms
                nc.vector.tensor_copy(sg[:, 33:34], nb[2 + g][:])  # k norms
                nc.sync.dma_start(out=cc_in[b, g], in_=sg[:])
            if use_collective:
                nc.gpsimd.collective_compute(
                    "AllReduce", mybir.AluOpType.add,
                    replica_groups=[list(range(NCORES))],
                    ins=[cc_in[b]], outs=[cc_out[b]])
            else:  # profiling build (no collectives allowed)
                nc.sync.dma_start(out=cc_out[b], in_=cc_in[b])
            emit_phase3(b)

    nc.compile()
    return nc


# ---------------------------------------------------------------- run

def _run(inputs, trace=False):
    from concourse.bass_utils import run_bass_kernel_spmd

    wd = _prep_weights(np.asarray(inputs["q_w"]), np.asarray(inputs["q_dw_w"]),
                       np.asarray(inputs["kv_w"]),
                       np.asarray(inputs["kv_dw_w"]),
                       np.asarray(inputs["proj_w"]),
                       np.asarray(inputs["temperature"]))
    slabs = _prep_slabs(np.asarray(inputs["low"], dtype=np.float32),
                        np.asarray(inputs["high"], dtype=np.float32))

    nc = _build_nc(PE_TAPS)
    in_maps = []
    for i in range(NCORES):
        m = dict(wd)
        m["high_s"] = slabs["high_s"][i]
        m["low_s"] = slabs["low_s"][i]
        in_maps.append(m)
    res = run_bass_kernel_spmd(nc, in_maps, list(range(NCORES)), trace=trace)

    out = np.empty((B, C, H, W), dtype=np.float32)
    for i in range(NCORES):
        o = res.results[i]["out"].reshape(B, C, RPC, W)
        out[:, :, RPC * i:RPC * (i + 1), :] = o
    return out, res


def kernel(**inputs):
    out, _ = _run(inputs, trace=False)
    return out


def kernel_profiled(**inputs):
    out, res = _run(inputs, trace=True)
    return out, res


# ---------------------------------------------------------------- numpy mock
# Exact-structure emulation of the device pipeline (validates shard/layout).

def numpy_pipeline(**inputs):
    low = np.asarray(inputs["low"], np.float32)
    high = np.asarray(inputs["high"], np.float32)
    wd = _prep_weights(np.asarray(inputs["q_w"]), np.asarray(inputs["q_dw_w"]),
                       np.asarray(inputs["kv_w"]),
                       np.asarray(inputs["kv_dw_w"]),
                       np.asarray(inputs["proj_w"]),
                       np.asarray(inputs["temperature"]))
    slabs = _prep_slabs(low, high)
    wq = wd["wq"].astype(np.float32).reshape(256, 256)
    wkv = wd["wkv"].astype(np.float32).reshape(256, 512)
    wproj = wd["wproj"].astype(np.float32).reshape(256, 256)
    dwvec = wd["dwvec"]
    tmpr = wd["tmpr"]

    Gd = np.zeros((B, 2, 128, 32), np.float32)
    qn2 = np.zeros((B, 2, 128), np.float32)
    kn2 = np.zeros((B, 2, 128), np.float32)
    vts = {}
    for i in range(NCORES):
        hi = slabs["high_s"][i].astype(np.float32).reshape(B, 256, SLABR, WE)
        lo = slabs["low_s"][i].astype(np.float32).reshape(B, 256, SLABR, WE)
        q1 = np.einsum("io,bihw->bohw", wq, hi)
        kv1 = np.einsum("io,bihw->bohw", wkv, lo)
        k1, v1 = kv1[:, :256], kv1[:, 256:]

        def dw(x, base_pt):
            o = np.zeros((B, 256, RPC, W), np.float32)
            for t, (dy, dx) in enumerate(TAPS):
                w9 = np.concatenate(
                    [dwvec[:, base_pt, t], dwvec[:, base_pt + 1, t]])
                o += (w9[None, :, None, None]
                      * x[:, :, 1 + dy:1 + dy + RPC, 1 + dx:1 + dx + W])
            return o

        qt, kt, vt = dw(q1, 0), dw(k1, 2), dw(v1, 4)
        vts[i] = vt
        for g in range(2):
            cs = slice(g * 128, (g + 1) * 128)
            qn2[:, g] += (qt[:, cs] ** 2).sum(axis=(2, 3))
            kn2[:, g] += (kt[:, cs] ** 2).sum(axis=(2, 3))
            for h in range(4):
                hs = slice(h * 32, (h + 1) * 32)
                qh = qt[:, cs][:, hs].reshape(B, 32, -1)
                kh = kt[:, cs][:, hs].reshape(B, 32, -1)
                Gd[:, g, hs, :] += np.einsum("bcn,bdn->bcd", qh, kh)

    out = np.empty((B, C, H, W), np.float32)
    for i in range(NCORES):
        o = np.zeros((B, 256, RPC, W), np.float32)
        for b in range(B):
            for g in range(2):
                cs = slice(g * 128, (g + 1) * 128)
                rq = 1.0 / np.sqrt(qn2[b, g])          # [128]
                rk = 1.0 / np.sqrt(kn2[b, g])          # [128]
                A = np.zeros((128, 128), np.float32)
                for h in range(4):
                    hs = slice(h * 32, (h + 1) * 32)
                    L = (Gd[b, g, hs, :] * rq[hs][:, None]
                         * tmpr[hs, g][:, None] * rk[hs][None, :])
                    L = L - L.max(axis=-1, keepdims=True)
                    E = np.exp(L)
                    A[hs, hs] = E / E.sum(axis=-1, keepdims=True)
                # M_g[d,o] = sum_c A[c,d] P[c,o] ; out += M^T v
                P = wproj[cs]                          # [128, 256]
                M = A.T @ P                            # [128 d, 256 o]
                vh = vts[i][b, cs].reshape(128, -1)    # [128 d, 3200]
                o[b] += np.einsum("do,dn->on", M, vh).reshape(256, RPC, W)
        out[:, :, RPC * i:RPC * (i + 1)] = o
    return out
